# revision 46
# baseline (speedup 1.0000x reference)
"""Trainium2 Bass kernel for nn_Attention_28604482191653.

Reference computation (B=4, S=2048, D=1024, H=4096, fp32):
    Q = x@Wq.T+bq; K = x@Wk.T+bk; V = x@Wv.T+bv     (per batch b)
    Q,K l2-normalized along features; sim = Q@K.T; attn = softmax(sim)
    out = attn@V; mlp: relu(out@W1.T+b1) -> relu(@W2.T+b2) -> @W3.T+b3

Sharding: 8 cores = (batch b, query half h); core c handles b=c//2 and
query rows [h*1024, (h+1)*1024) with h=c%2.  K/V are recomputed per
core pair (no collectives).  All matmul operands are bf16 (PSUM
accumulates fp32; measured end-to-end rel err ~4e-3 vs the 2e-2 gate),
which halves DMA traffic and SBUF footprint vs fp32r at the same PE
rate.  Every activation stays SBUF-resident (no DRAM scratch):

  xT [f,s] (own query half first: softmax is permutation-invariant over
  key positions, so per-core column order avoids a per-core program)
  -> QT/KT feature-major (+bias via ACT, row norms via squares +
  ones-matmul; 1/||q|| broadcast by K=1 matmul, 1/||k|| folded into the
  exp scale), V natural [s,d] (x-stationary), simT=[k,q] -> exp -> PT
  bf16, denominator via ones-matmuls, PV with V-stationary -> attention
  out feature-major, 3-layer MLP feature-major; final layer emits
  out_pm [d, rows] and the HOST transposes (so b3 is a per-partition
  ACT bias and no transposing DMA is needed).

SBUF reuse via same-tag pool slots: xT->h2b, kt->h2a, qt->h1,
wv->oT->w3 stream.  W2 is streamed twice (once per 512-row query
slice) on the sync HWDGE ring to stay inside SBUF.  PE is warmed with
dummy matmuls during the initial x DMA so HAM reaches 8/8 before real
work; ACT tables preload early; row-reductions (norms, softmax
denominators) are batched ones-matmuls to minimize stationary-switch
bubbles; reciprocals run on DVE only after broadcast/transpose puts
them off the critical path.  Measured ~551us on HW (baseline 677us),
PE busy 96%, pitch ~226ns vs the 216ns N=512 issue floor.
"""

import numpy as np

B, S, D, H = 4, 2048, 1024, 4096
P = 128
NS = 512
QROWS = S // 2
N_CORES = 8
DK = D // P     # 8  feature tiles of d_model
SK = S // P     # 16 key-position tiles
HK = H // P     # 32 hidden tiles
QS = QROWS // NS   # 2 query column slices per core
SNS = S // NS      # 4 key column slices
WARM_N = 112       # warmup matmuls (N=128) during initial DMA

_BUILT = None
_LAST_INSTS = None


def _build():
    import concourse.bass as bass
    import concourse.tile as tile
    from concourse import bacc, mybir

    F32 = mybir.dt.float32
    F32R = mybir.dt.float32r
    BF16 = mybir.dt.bfloat16
    ALU = mybir.AluOpType
    AF = mybir.ActivationFunctionType

    nc = bacc.Bacc("TRN2", target_bir_lowering=False, debug=False)

    # ---- I/O ----
    xTd = nc.dram_tensor("xTd", [D, S], BF16, kind="ExternalInput")
    wq_pm = nc.dram_tensor("wq_pm", [D, D], BF16, kind="ExternalInput")
    wk_pm = nc.dram_tensor("wk_pm", [D, D], BF16, kind="ExternalInput")
    wvT = nc.dram_tensor("wvT", [D, D], BF16, kind="ExternalInput")
    w1_pm = nc.dram_tensor("w1_pm", [D, D], BF16, kind="ExternalInput")
    w2_pm = nc.dram_tensor("w2_pm", [H, D], BF16, kind="ExternalInput")
    w3_pm = nc.dram_tensor("w3_pm", [D, H], BF16, kind="ExternalInput")
    bq_col = nc.dram_tensor("bq_col", [P, DK], F32, kind="ExternalInput")
    bk_col = nc.dram_tensor("bk_col", [P, DK], F32, kind="ExternalInput")
    b1_col = nc.dram_tensor("b1_col", [P, DK], F32, kind="ExternalInput")
    b2_col = nc.dram_tensor("b2_col", [P, HK], F32, kind="ExternalInput")
    b3_col = nc.dram_tensor("b3_col", [P, DK], F32, kind="ExternalInput")
    bv_row = nc.dram_tensor("bv_row", [1, D], BF16, kind="ExternalInput")
    out_pm = nc.dram_tensor("out_pm", [D, QROWS], F32, kind="ExternalOutput")

    def bcast_ap(sliced, n):
        """AP replicating a [1, n] DRAM slice across all 128 partitions."""
        return bass.AP(tensor=sliced.tensor, offset=sliced.offset,
                       ap=[[0, P], [1, n]])

    with tile.TileContext(nc, pool_alloc_mode="queue") as tc:
        dram = tc.alloc_tile_pool(name="dram", bufs=1, space="DRAM")
        rk_scr = dram.tile([1, S], F32)

        constp = tc.alloc_tile_pool(name="const", bufs=1)
        bigA = tc.alloc_tile_pool(name="bigA", bufs=1)    # xT -> h2b
        qtp = tc.alloc_tile_pool(name="qtp", bufs=1)      # qt -> h1
        bigB = tc.alloc_tile_pool(name="bigB", bufs=1)    # kt -> h2a
        bigC = tc.alloc_tile_pool(name="bigC", bufs=1)    # v
        # wv slabs -> oTa/oTb -> w3 stream all share two 8KB/part slots
        oTp = tc.alloc_tile_pool(name="oTp", bufs=2)
        streamp = tc.alloc_tile_pool(name="streamp", bufs=6)  # wq/wk/w2 blocks
        w1p = tc.alloc_tile_pool(name="w1p", bufs=8)      # w1 blocks, resident
        workp = tc.alloc_tile_pool(name="workp", bufs=1)  # sq/rows/ost/pt

        pp = tc.alloc_tile_pool(name="pp", bufs=4, space="PSUM")   # mm groups
        sp = tc.alloc_tile_pool(name="sp", bufs=3, space="PSUM")   # row sums
        bp = tc.alloc_tile_pool(name="bp", bufs=1, space="PSUM")   # bcast/denom

        # ---- constants: ones/warm via memset (no DMA dependency) ----
        ones_bf = constp.tile([P, 1], BF16)
        nc.vector.memset(ones_bf[:], 1.0)
        onesr_f = constp.tile([1, P], F32)
        nc.vector.memset(onesr_f[:], 1.0)
        onesr = constp.tile([1, P], F32R)
        nc.scalar.copy(onesr[:], onesr_f[:])
        warm = constp.tile([P, P], BF16)
        nc.vector.memset(warm[:], 1.0)
        bqc = constp.tile([P, DK], F32)
        bkc = constp.tile([P, DK], F32)
        b1c = constp.tile([P, DK], F32)
        b2c = constp.tile([P, HK], F32)
        b3c = constp.tile([P, DK], F32)
        bvb = constp.tile([P, D], BF16)
        rk_col = constp.tile([P, SK], F32)
        rqb = constp.tile([P, QROWS], F32)
        rsb = constp.tile([P, QS, NS], F32)
        warm_sink = constp.tile([1, P], F32)

        # ---- PE warmup during initial DMA (HAM to 8/8 before real MMs) ----
        warm_ps = bp.tile([1, P], F32, tag="bc")
        for _ in range(WARM_N):
            nc.tensor.matmul(warm_ps[:], ones_bf[:], warm[:, :],
                             start=True, stop=True)
        nc.vector.tensor_copy(out=warm_sink[:], in_=warm_ps[:])

        # ---- xT load: own query half first, all on the fast sync ring
        # (the gpsimd SWDGE ring takes ~30us to boot).  Four big merged
        # transfers -- per-chunk sem round trips dominate small ones.
        xt = bigA.tile([P, DK, S], BF16, tag="bigA")
        for half, sl in ((0, slice(0, QROWS)), (1, slice(QROWS, S))):
            for kk4 in range(0, DK, 4):
                nc.sync.dma_start(
                    out=xt[:, kk4:kk4 + 4, sl],
                    in_=xTd[kk4 * P:(kk4 + 4) * P,
                            half * QROWS:(half + 1) * QROWS].rearrange(
                        "(kk p) n -> p kk n", p=P))
        nc.gpsimd.dma_start(out=bvb[:], in_=bcast_ap(bv_row[0:1, :], D))

        def wblock(src, i, eng, pool=None, tag="wblk"):
            """[P, DK, P] stationary block i of a PE-major weight matrix."""
            pool = streamp if pool is None else pool
            w_sb = pool.tile([P, DK, P], BF16, tag=tag, name=f"wb{i}")
            eng.dma_start(
                out=w_sb[:],
                in_=src[i * P:(i + 1) * P, :].rearrange(
                    "p (kk n) -> p kk n", kk=DK))
            return w_sb

        # =============== Q projection (own half, feature-major) ===========
        qt = qtp.tile([P, DK, QROWS], BF16, tag="qtp")
        ssq = [sp.tile([1, NS], F32, tag="sum", name=f"ssq{i}")
               for i in range(QS)]
        pend = []

        def flush_pend(lim):
            while len(pend) > lim:
                pend.pop(0)()

        # first two weight blocks lead the scalar ring; bias columns follow
        wq_sb = [wblock(wq_pm, 0, nc.scalar), wblock(wq_pm, 1, nc.scalar)]
        nc.scalar.dma_start(out=bqc[:], in_=bq_col[:, :])
        nc.scalar.dma_start(out=bkc[:], in_=bk_col[:, :])
        nc.scalar.dma_start(out=b1c[:], in_=b1_col[:, :])
        nc.scalar.dma_start(out=b2c[:], in_=b2_col[:, :])
        nc.scalar.dma_start(out=b3c[:], in_=b3_col[:, :])
        # Preload ACT interpolation tables (no data deps -> run ~t=5us).
        dummy_sink = constp.tile([1, 1], F32)
        for fn in (AF.Identity, AF.Exp, AF.Sqrt, AF.Relu):
            nc.scalar.activation(dummy_sink[0:1, 0:1], warm[0:1, 0:1], fn)
        for m in range(DK):
            if m + 2 < DK:
                wq_sb.append(wblock(wq_pm, m + 2, nc.scalar))
            wcur = wq_sb[m]
            for nn in range(QS):
                sl = slice(nn * NS, (nn + 1) * NS)
                ps = pp.tile([P, NS], F32, tag="mm")
                for kk in range(DK):
                    nc.tensor.matmul(ps[:], wcur[:, kk, :], xt[:, kk, sl],
                                     start=(kk == 0), stop=(kk == DK - 1))
                nc.scalar.activation(qt[:, m, sl], ps[:], AF.Identity,
                                     bias=bqc[:, m:m + 1])
                sq = workp.tile([P, NS], BF16, tag="sq", bufs=10)
                nc.vector.tensor_mul(sq[:], qt[:, m, sl], qt[:, m, sl])

                def qsum(m=m, nn=nn, sq=sq):
                    nc.tensor.matmul(ssq[nn][:], ones_bf[:], sq[:],
                                     start=(m == 0), stop=(m == DK - 1))
                pend.append(qsum)
                flush_pend(2)
        flush_pend(0)
        # ||q_row|| on ACT right away (frees the sum-psum slots); the PE
        # broadcast + DVE reciprocal + normalize are deferred into the K
        # loop.  Reciprocal runs on the broadcast [128, NS] PSUM tile --
        # same per-lane serial cost as [1, NS], but off the critical path.
        rq_rowr = []
        for nn in range(QS):
            r = workp.tile([1, NS], F32R, tag="rowr", bufs=2,
                           name=f"rqr{nn}")
            nc.scalar.activation(r[:], ssq[nn][:], AF.Sqrt)
            rq_rowr.append(r)

        def q_norm_bcast(nn):
            sl = slice(nn * NS, (nn + 1) * NS)
            pb = bp.tile([P, NS], F32, tag="bc")
            nc.tensor.matmul(pb[:], onesr[:], rq_rowr[nn][:],
                             start=True, stop=True)
            nc.vector.reciprocal(rqb[:, sl], pb[:])

        def q_norm_mul():
            for m in range(DK):
                for nn in range(QS):
                    sl = slice(nn * NS, (nn + 1) * NS)
                    nc.vector.tensor_mul(qt[:, m, sl], qt[:, m, sl],
                                         rqb[:, sl])

        # =============== K projection (full S, feature-major) =============
        # nn-outer so only one ss sum tile is live at a time (sp bufs=3);
        # wk blocks are re-streamed per nn pass.  q_normalize pieces are
        # spread into the early passes so their DVE cost hides.
        kt = bigB.tile([P, DK, S], BF16, tag="bigB")
        wk_seq = [wblock(wk_pm, 0, nc.scalar), wblock(wk_pm, 1, nc.scalar)]
        kidx = 0
        sq_pass = []
        for nn in range(SNS):
            sl = slice(nn * NS, (nn + 1) * NS)
            ss_t = sp.tile([1, NS], F32, tag="sum", name=f"ss{nn}")
            for m in range(DK):
                if kidx + 2 < SNS * DK:
                    wk_seq.append(wblock(wk_pm, (kidx + 2) % DK, nc.scalar))
                wcur = wk_seq[kidx]
                if kidx == 4:
                    q_norm_bcast(0)
                elif kidx == 6:
                    q_norm_bcast(1)
                elif kidx == 10:
                    q_norm_mul()
                ps = pp.tile([P, NS], F32, tag="mm")
                for kk in range(DK):
                    nc.tensor.matmul(ps[:], wcur[:, kk, :], xt[:, kk, sl],
                                     start=(kk == 0), stop=(kk == DK - 1))
                nc.scalar.activation(kt[:, m, sl], ps[:], AF.Identity,
                                     bias=bkc[:, m:m + 1])
                sq = workp.tile([P, NS], BF16, tag="sq", bufs=10,
                                name=f"sqk{nn}_{m}")
                nc.vector.tensor_mul(sq[:], kt[:, m, sl], kt[:, m, sl])
                sq_pass.append(sq)
                kidx += 1
            # batched row-sum matmuls: one stationary transition per pass
            # instead of eight interleaved ones
            for m, sq in enumerate(sq_pass):
                nc.tensor.matmul(ss_t[:], ones_bf[:], sq[:],
                                 start=(m == 0), stop=(m == DK - 1))
            sq_pass.clear()
            # ||k_row|| for this slice as soon as its sums stop
            rk_row = workp.tile([1, NS], F32, tag="row", bufs=2)
            nc.scalar.activation(rk_row[:], ss_t[:], AF.Sqrt)
            nc.scalar.dma_start(out=rk_scr[0:1, nn * NS:(nn + 1) * NS],
                                in_=rk_row[:])
        # transpose to partitions, then reciprocal on [128, SK] (16/lane)
        rk_flat = rk_scr[0:1, :]
        nc.scalar.dma_start(
            out=rk_col[:],
            in_=bass.AP(tensor=rk_flat.tensor, offset=rk_flat.offset,
                        ap=[[1, P], [P, SK]]))
        nc.vector.reciprocal(rk_col[:], rk_col[:])

        # w1 blocks: own pool, all 8 resident well before MLP1
        w1_sb = [wblock(w1_pm, m, nc.scalar, pool=w1p, tag="w1")
                 for m in range(DK)]

        # =============== V projection (natural [s, d], x-stationary) ======
        v_sb = bigC.tile([P, SK, D], BF16, tag="bigC")
        wv_sl = []
        for dn in range(2):
            wv = oTp.tile([P, DK, NS], BF16, tag="oT", name=f"wv{dn}")
            nc.sync.dma_start(
                out=wv[:],
                in_=wvT[:, dn * NS:(dn + 1) * NS].rearrange(
                    "(kk p) n -> p kk n", p=P))
            wv_sl.append(wv)
        for dn in range(2):
            dsl = slice(dn * NS, (dn + 1) * NS)
            for st in range(SK):
                ps = pp.tile([P, NS], F32, tag="mm")
                for kk in range(DK):
                    nc.tensor.matmul(
                        ps[:], xt[:, kk, st * P:(st + 1) * P],
                        wv_sl[dn][:, kk, :],
                        start=(kk == 0), stop=(kk == DK - 1))
                nc.vector.scalar_tensor_tensor(
                    out=v_sb[:, st, dsl], in0=ps[:], scalar=1.0,
                    in1=bvb[:, dsl], op0=ALU.mult, op1=ALU.add)

        # =============== attention + MLP1 (interleaved issue) =============
        oTa = oTp.tile([P, DK // 2, QROWS], BF16, tag="oT", name="oTa")
        oTb = oTp.tile([P, DK // 2, QROWS], BF16, tag="oT", name="oTb")

        def oT(m, sl):
            t = oTa if m < DK // 2 else oTb
            return t[:, m % (DK // 2), sl]

        h1 = qtp.tile([P, DK, QROWS], BF16, tag="qtp", name="h1")

        def attention(qs):
            qsl = slice(qs * NS, (qs + 1) * NS)
            pt = workp.tile([P, SK, NS], BF16, tag="pt", bufs=1,
                            name=f"pt{qs}")
            s_ps = sp.tile([1, NS], F32, tag="sum", name=f"sden{qs}")
            for kkt in range(SK):
                ps = pp.tile([P, NS], F32, tag="mm")
                for kk in range(DK):
                    nc.tensor.matmul(
                        ps[:], kt[:, kk, kkt * P:(kkt + 1) * P],
                        qt[:, kk, qsl],
                        start=(kk == 0), stop=(kk == DK - 1))
                nc.scalar.activation(pt[:, kkt, :], ps[:], AF.Exp,
                                     scale=rk_col[:, kkt:kkt + 1])

            def pv_mms(m, pt=pt):
                po = pp.tile([P, NS], F32, tag="mm")
                for kkt in range(SK):
                    nc.tensor.matmul(po[:], v_sb[:, kkt, m * P:(m + 1) * P],
                                     pt[:, kkt, :],
                                     start=(kkt == 0), stop=(kkt == SK - 1))
                return po
            # PV m=0/1 don't need rsb -- only the DVE scale-out does -- so
            # the batched denominator sums, K=1 broadcast, and reciprocal
            # all hide under them.
            po0 = pv_mms(0)
            for kkt in range(SK):
                nc.tensor.matmul(s_ps[:], ones_bf[:], pt[:, kkt, :],
                                 start=(kkt == 0), stop=(kkt == SK - 1))
            po1 = pv_mms(1)
            s_rowr = workp.tile([1, NS], F32R, tag="rowr", bufs=2)
            nc.scalar.activation(s_rowr[:], s_ps[:], AF.Identity)
            pb = bp.tile([P, NS], F32, tag="bc")
            nc.tensor.matmul(pb[:], onesr[:], s_rowr[:], start=True,
                             stop=True)
            nc.vector.reciprocal(rsb[:, qs, :], pb[:])
            nc.vector.tensor_mul(oT(0, qsl), po0[:], rsb[:, qs, :])
            nc.vector.tensor_mul(oT(1, qsl), po1[:], rsb[:, qs, :])
            for m in range(2, DK):
                po = pv_mms(m)
                nc.vector.tensor_mul(oT(m, qsl), po[:], rsb[:, qs, :])

        def mlp1(nn):
            sl = slice(nn * NS, (nn + 1) * NS)
            for m in range(DK):
                ps = pp.tile([P, NS], F32, tag="mm")
                for kk in range(DK):
                    nc.tensor.matmul(ps[:], w1_sb[m][:, kk, :], oT(kk, sl),
                                     start=(kk == 0), stop=(kk == DK - 1))
                nc.scalar.activation(h1[:, m, sl], ps[:], AF.Relu,
                                     bias=b1c[:, m:m + 1])

        attention(0)
        attention(1)
        mlp1(0)
        mlp1(1)

        # =============== MLP2 (h2 resident bf16; W2 streamed per slice) ===
        # h2 halves land in the slots kt and xT vacated (same 32KB/part).
        h2a = bigB.tile([P, HK // 2, QROWS], BF16, tag="bigB", name="h2a")
        h2b = bigA.tile([P, HK // 2, QROWS], BF16, tag="bigA", name="h2b")

        def h2(ht, sl):
            t = h2a if ht < HK // 2 else h2b
            return t[:, ht % (HK // 2), sl]

        for nn in range(QS):
            sl = slice(nn * NS, (nn + 1) * NS)
            w2_sb = [wblock(w2_pm, 0, nc.sync), wblock(w2_pm, 1, nc.sync)]
            for ht in range(HK):
                if ht + 2 < HK:
                    w2_sb.append(wblock(w2_pm, ht + 2, nc.sync))
                wcur = w2_sb[ht]
                ps = pp.tile([P, NS], F32, tag="mm")
                for kk in range(DK):
                    nc.tensor.matmul(ps[:], wcur[:, kk, :], h1[:, kk, sl],
                                     start=(kk == 0), stop=(kk == DK - 1))
                nc.scalar.activation(h2(ht, sl), ps[:], AF.Relu,
                                     bias=b2c[:, ht:ht + 1])
                w2_sb[ht] = None

        # =============== MLP3 (feature-major out; host transposes) ========
        w3_sb = []

        def w3block(dt):
            w3t = oTp.tile([P, HK, P], BF16, tag="oT", name=f"w3b{dt}")
            nc.gpsimd.dma_start(
                out=w3t[:],
                in_=w3_pm[dt * P:(dt + 1) * P, :].rearrange(
                    "p (ht n) -> p ht n", ht=HK))
            return w3t

        w3_sb = [w3block(0), w3block(1)]
        for dt in range(DK):
            if dt + 2 < DK:
                w3_sb.append(w3block(dt + 2))
            wcur = w3_sb[dt]
            for nn in range(QS):
                sl = slice(nn * NS, (nn + 1) * NS)
                ps = pp.tile([P, NS], F32, tag="mm")
                for ht in range(HK):
                    nc.tensor.matmul(ps[:], wcur[:, ht, :], h2(ht, sl),
                                     start=(ht == 0), stop=(ht == HK - 1))
                ost = workp.tile([P, NS], F32, tag="ost", bufs=2)
                nc.scalar.activation(ost[:], ps[:], AF.Identity,
                                     bias=b3c[:, dt:dt + 1])
                nc.sync.dma_start(
                    out=out_pm[dt * P:(dt + 1) * P, sl], in_=ost[:])
            w3_sb[dt] = None

        for pool in (bp, sp, pp, workp, w1p, streamp, oTp, bigC,
                     bigB, qtp, bigA, constp, dram):
            pool.release()

    nc.compile()
    return nc


def _get_built():
    global _BUILT
    if _BUILT is None:
        _BUILT = _build()
    return _BUILT


def _pe_major(w, rows, cols):
    """[rows, cols] -> PE-major: block (m) holds lhsT [in-f part, out-f]."""
    return np.ascontiguousarray(
        w.reshape(rows // P, P, cols // P, P).transpose(2, 1, 0, 3)
        .reshape(cols, rows))


def _host_prep(inputs):
    import ml_dtypes
    bf16 = ml_dtypes.bfloat16
    f32 = np.float32

    def bf(a):
        return np.ascontiguousarray(np.asarray(a, f32).astype(bf16))

    x = np.asarray(inputs["x"], f32)
    shared = {
        "wq_pm": _pe_major(bf(inputs["Wq"]).T, D, D),
        "wk_pm": _pe_major(bf(inputs["Wk"]).T, D, D),
        "wvT": np.ascontiguousarray(bf(inputs["Wv"]).T),
        "w1_pm": _pe_major(bf(inputs["W1"]).T, D, D),
        "w2_pm": _pe_major(bf(inputs["W2"]).T, D, H),
        "w3_pm": _pe_major(bf(inputs["W3"]).T, H, D),
        "bq_col": np.ascontiguousarray(
            np.asarray(inputs["bq"], f32).reshape(DK, P).T),
        "bk_col": np.ascontiguousarray(
            np.asarray(inputs["bk"], f32).reshape(DK, P).T),
        "b1_col": np.ascontiguousarray(
            np.asarray(inputs["b1"], f32).reshape(DK, P).T),
        "b2_col": np.ascontiguousarray(
            np.asarray(inputs["b2"], f32).reshape(HK, P).T),
        "b3_col": np.ascontiguousarray(
            np.asarray(inputs["b3"], f32).reshape(DK, P).T),
        "bv_row": bf(inputs["bv"]).reshape(1, D),
    }
    in_maps = []
    for c in range(N_CORES):
        b, h = c // 2, c % 2
        m = dict(shared)
        xb = bf(x[b]).T  # [D, S]
        if h == 0:
            m["xTd"] = np.ascontiguousarray(xb)
        else:
            m["xTd"] = np.ascontiguousarray(
                np.concatenate([xb[:, QROWS:], xb[:, :QROWS]], axis=1))
        in_maps.append(m)
    return in_maps


def run_kernel(inputs, trace=False):
    """Returns (output [B,S,D] f32, exec_time_ns or None)."""
    from concourse.bass_utils import run_bass_kernel_spmd

    if trace:
        _install_ntff_hook()
    nc = _get_built()
    in_maps = _host_prep(inputs)
    res = run_bass_kernel_spmd(
        nc, in_maps, core_ids=list(range(N_CORES)), trace=trace)
    global _LAST_INSTS
    if res.instructions_and_trace is not None:
        _LAST_INSTS = res.instructions_and_trace[0]
    outp = np.empty((B, S, D), np.float32)
    for c in range(N_CORES):
        b, h = c // 2, c % 2
        outp[b, h * QROWS:(h + 1) * QROWS, :] = res.results[c]["out_pm"].T
    return outp, res.exec_time_ns


def kernel(**inputs):
    return run_kernel(inputs, trace=False)[0]


def _install_ntff_hook():
    """Register the axon NTFF profiling hook (used only when trace=True)."""
    import sys
    import types

    if "antenv.axon_hooks" in sys.modules:
        return
    try:
        import antenv
        from trn_agent_boot.trn_boot import _ntff_profile_via_ctypes
    except ImportError:
        return
    hooks = types.ModuleType("antenv.axon_hooks")
    _h = [_ntff_profile_via_ctypes("/opt/axon/libaxon_pjrt.so")]
    hooks.set_axon_ntff_profile_hook = lambda h: _h.__setitem__(0, h)
    hooks.get_axon_ntff_profile_hook = lambda: _h[0]
    sys.modules["antenv.axon_hooks"] = hooks
    antenv.axon_hooks = hooks


# revision 47
# speedup vs baseline: 1.0104x; 1.0104x over previous
"""Trainium2 Bass kernel for nn_Attention_28604482191653.

Reference computation (B=4, S=2048, D=1024, H=4096, fp32):
    Q = x@Wq.T+bq; K = x@Wk.T+bk; V = x@Wv.T+bv     (per batch b)
    Q,K l2-normalized along features; sim = Q@K.T; attn = softmax(sim)
    out = attn@V; mlp: relu(out@W1.T+b1) -> relu(@W2.T+b2) -> @W3.T+b3

Sharding: 8 cores = (batch b, query half h); core c handles b=c//2 and
query rows [h*1024, (h+1)*1024) with h=c%2.  K/V are recomputed per
core pair (no collectives).  All matmul operands are bf16 (PSUM
accumulates fp32; measured end-to-end rel err ~4e-3 vs the 2e-2 gate),
which halves DMA traffic and SBUF footprint vs fp32r at the same PE
rate.  Every activation stays SBUF-resident (no DRAM scratch):

  xT [f,s] (own query half first: softmax is permutation-invariant over
  key positions, so per-core column order avoids a per-core program)
  -> QT/KT feature-major (+bias via ACT, row norms via squares +
  ones-matmul; 1/||q|| broadcast by K=1 matmul, 1/||k|| folded into the
  exp scale), V natural [s,d] (x-stationary), simT=[k,q] -> exp -> PT
  bf16, denominator via ones-matmuls, PV with V-stationary -> attention
  out feature-major, 3-layer MLP feature-major; final layer emits
  out_pm [d, rows] and the HOST transposes (so b3 is a per-partition
  ACT bias and no transposing DMA is needed).

SBUF reuse via same-tag pool slots: xT->h2b, kt->h2a, qt->h1,
wv->oT->w3 stream.  W2 is streamed twice (once per 512-row query
slice) on the sync HWDGE ring to stay inside SBUF.  PE is warmed with
dummy matmuls during the initial x DMA so HAM reaches 8/8 before real
work; ACT tables preload early; row-reductions (norms, softmax
denominators) are batched ones-matmuls to minimize stationary-switch
bubbles; reciprocals run on DVE only after broadcast/transpose puts
them off the critical path.  Measured ~551us on HW (baseline 677us),
PE busy 96%, pitch ~226ns vs the 216ns N=512 issue floor.
"""

import numpy as np

B, S, D, H = 4, 2048, 1024, 4096
P = 128
NS = 512
QROWS = S // 2
N_CORES = 8
DK = D // P     # 8  feature tiles of d_model
SK = S // P     # 16 key-position tiles
HK = H // P     # 32 hidden tiles
QS = QROWS // NS   # 2 query column slices per core
SNS = S // NS      # 4 key column slices
WARM_N = 112       # warmup matmuls (N=128) during initial DMA

_BUILT = None
_LAST_INSTS = None


def _build():
    import concourse.bass as bass
    import concourse.tile as tile
    from concourse import bacc, mybir

    F32 = mybir.dt.float32
    F32R = mybir.dt.float32r
    BF16 = mybir.dt.bfloat16
    ALU = mybir.AluOpType
    AF = mybir.ActivationFunctionType

    nc = bacc.Bacc("TRN2", target_bir_lowering=False, debug=False)

    # ---- I/O ----
    xTd = nc.dram_tensor("xTd", [D, S], BF16, kind="ExternalInput")
    wq_pm = nc.dram_tensor("wq_pm", [D, D], BF16, kind="ExternalInput")
    wk_pm = nc.dram_tensor("wk_pm", [D, D], BF16, kind="ExternalInput")
    wvT = nc.dram_tensor("wvT", [D, D], BF16, kind="ExternalInput")
    w1_pm = nc.dram_tensor("w1_pm", [D, D], BF16, kind="ExternalInput")
    w2_pm = nc.dram_tensor("w2_pm", [H, D], BF16, kind="ExternalInput")
    w3_pm = nc.dram_tensor("w3_pm", [D, H], BF16, kind="ExternalInput")
    bq_col = nc.dram_tensor("bq_col", [P, DK], F32, kind="ExternalInput")
    bk_col = nc.dram_tensor("bk_col", [P, DK], F32, kind="ExternalInput")
    b1_col = nc.dram_tensor("b1_col", [P, DK], F32, kind="ExternalInput")
    b2_col = nc.dram_tensor("b2_col", [P, HK], F32, kind="ExternalInput")
    b3_col = nc.dram_tensor("b3_col", [P, DK], F32, kind="ExternalInput")
    bv_row = nc.dram_tensor("bv_row", [1, D], BF16, kind="ExternalInput")
    out_pm = nc.dram_tensor("out_pm", [D, QROWS], F32, kind="ExternalOutput")

    def bcast_ap(sliced, n):
        """AP replicating a [1, n] DRAM slice across all 128 partitions."""
        return bass.AP(tensor=sliced.tensor, offset=sliced.offset,
                       ap=[[0, P], [1, n]])

    with tile.TileContext(nc, pool_alloc_mode="queue") as tc:
        dram = tc.alloc_tile_pool(name="dram", bufs=1, space="DRAM")
        rk_scr = dram.tile([1, S], F32)

        constp = tc.alloc_tile_pool(name="const", bufs=1)
        bigA = tc.alloc_tile_pool(name="bigA", bufs=1)    # xT -> h2b
        qtp = tc.alloc_tile_pool(name="qtp", bufs=1)      # qt -> h1
        bigB = tc.alloc_tile_pool(name="bigB", bufs=1)    # kt -> h2a
        bigC = tc.alloc_tile_pool(name="bigC", bufs=1)    # v
        # wv slabs -> oTa/oTb -> w3 stream all share two 8KB/part slots
        oTp = tc.alloc_tile_pool(name="oTp", bufs=2)
        streamp = tc.alloc_tile_pool(name="streamp", bufs=6)  # wq/wk/w2 blocks
        w1p = tc.alloc_tile_pool(name="w1p", bufs=8)      # w1 blocks, resident
        workp = tc.alloc_tile_pool(name="workp", bufs=1)  # sq/rows/ost/pt

        pp = tc.alloc_tile_pool(name="pp", bufs=4, space="PSUM")   # mm groups
        sp = tc.alloc_tile_pool(name="sp", bufs=3, space="PSUM")   # row sums
        bp = tc.alloc_tile_pool(name="bp", bufs=1, space="PSUM")   # bcast/denom

        # ---- constants: ones/warm via memset (no DMA dependency) ----
        ones_bf = constp.tile([P, 1], BF16)
        nc.vector.memset(ones_bf[:], 1.0)
        onesr_f = constp.tile([1, P], F32)
        nc.vector.memset(onesr_f[:], 1.0)
        onesr = constp.tile([1, P], F32R)
        nc.scalar.copy(onesr[:], onesr_f[:])
        warm = constp.tile([P, P], BF16)
        nc.vector.memset(warm[:], 1.0)
        bqc = constp.tile([P, DK], F32)
        bkc = constp.tile([P, DK], F32)
        b1c = constp.tile([P, DK], F32)
        b2c = constp.tile([P, HK], F32)
        b3c = constp.tile([P, DK], F32)
        bvb = constp.tile([P, D], BF16)
        rk_col = constp.tile([P, SK], F32)
        rqb = constp.tile([P, QROWS], F32)
        rsb = constp.tile([P, QS, NS], F32)
        warm_sink = constp.tile([1, P], F32)

        # ---- PE warmup during initial DMA (HAM to 8/8 before real MMs) ----
        warm_ps = bp.tile([1, P], F32, tag="bc")
        for _ in range(WARM_N):
            nc.tensor.matmul(warm_ps[:], ones_bf[:], warm[:, :],
                             start=True, stop=True)
        nc.vector.tensor_copy(out=warm_sink[:], in_=warm_ps[:])

        # ---- xT load: own query half first, all on the fast sync ring
        # (the gpsimd SWDGE ring takes ~30us to boot).  Four big merged
        # transfers -- per-chunk sem round trips dominate small ones.
        xt = bigA.tile([P, DK, S], BF16, tag="bigA")
        for half, sl in ((0, slice(0, QROWS)), (1, slice(QROWS, S))):
            for kk4 in range(0, DK, 4):
                nc.sync.dma_start(
                    out=xt[:, kk4:kk4 + 4, sl],
                    in_=xTd[kk4 * P:(kk4 + 4) * P,
                            half * QROWS:(half + 1) * QROWS].rearrange(
                        "(kk p) n -> p kk n", p=P))
        nc.gpsimd.dma_start(out=bvb[:], in_=bcast_ap(bv_row[0:1, :], D))

        def wblock(src, i, eng, pool=None, tag="wblk"):
            """[P, DK, P] stationary block i of a PE-major weight matrix."""
            pool = streamp if pool is None else pool
            w_sb = pool.tile([P, DK, P], BF16, tag=tag, name=f"wb{i}")
            eng.dma_start(
                out=w_sb[:],
                in_=src[i * P:(i + 1) * P, :].rearrange(
                    "p (kk n) -> p kk n", kk=DK))
            return w_sb

        # =============== Q projection (own half, feature-major) ===========
        qt = qtp.tile([P, DK, QROWS], BF16, tag="qtp")
        ssq = [sp.tile([1, NS], F32, tag="sum", name=f"ssq{i}")
               for i in range(QS)]
        pend = []

        def flush_pend(lim):
            while len(pend) > lim:
                pend.pop(0)()

        # first two weight blocks lead the scalar ring; bias columns follow
        wq_sb = [wblock(wq_pm, 0, nc.scalar), wblock(wq_pm, 1, nc.scalar)]
        nc.scalar.dma_start(out=bqc[:], in_=bq_col[:, :])
        nc.scalar.dma_start(out=bkc[:], in_=bk_col[:, :])
        nc.scalar.dma_start(out=b1c[:], in_=b1_col[:, :])
        nc.scalar.dma_start(out=b2c[:], in_=b2_col[:, :])
        nc.scalar.dma_start(out=b3c[:], in_=b3_col[:, :])
        # Preload ACT interpolation tables (no data deps -> run ~t=5us).
        dummy_sink = constp.tile([1, 1], F32)
        for fn in (AF.Identity, AF.Exp, AF.Sqrt, AF.Relu):
            nc.scalar.activation(dummy_sink[0:1, 0:1], warm[0:1, 0:1], fn)
        for m in range(DK):
            if m + 2 < DK:
                wq_sb.append(wblock(wq_pm, m + 2, nc.scalar))
            wcur = wq_sb[m]
            for nn in range(QS):
                sl = slice(nn * NS, (nn + 1) * NS)
                ps = pp.tile([P, NS], F32, tag="mm")
                for kk in range(DK):
                    nc.tensor.matmul(ps[:], wcur[:, kk, :], xt[:, kk, sl],
                                     start=(kk == 0), stop=(kk == DK - 1))
                nc.scalar.activation(qt[:, m, sl], ps[:], AF.Identity,
                                     bias=bqc[:, m:m + 1])
                sq = workp.tile([P, NS], BF16, tag="sq", bufs=10)
                nc.vector.tensor_mul(sq[:], qt[:, m, sl], qt[:, m, sl])

                def qsum(m=m, nn=nn, sq=sq):
                    nc.tensor.matmul(ssq[nn][:], ones_bf[:], sq[:],
                                     start=(m == 0), stop=(m == DK - 1))
                pend.append(qsum)
                flush_pend(2)
        flush_pend(0)
        # ||q_row|| on ACT right away (frees the sum-psum slots); the PE
        # broadcast + DVE reciprocal + normalize are deferred into the K
        # loop.  Reciprocal runs on the broadcast [128, NS] PSUM tile --
        # same per-lane serial cost as [1, NS], but off the critical path.
        rq_rowr = []
        for nn in range(QS):
            r = workp.tile([1, NS], F32R, tag="rowr", bufs=2,
                           name=f"rqr{nn}")
            nc.scalar.activation(r[:], ssq[nn][:], AF.Sqrt)
            rq_rowr.append(r)

        def q_norm_bcast(nn):
            sl = slice(nn * NS, (nn + 1) * NS)
            pb = bp.tile([P, NS], F32, tag="bc")
            nc.tensor.matmul(pb[:], onesr[:], rq_rowr[nn][:],
                             start=True, stop=True)
            nc.vector.reciprocal(rqb[:, sl], pb[:])

        def q_norm_mul():
            for m in range(DK):
                for nn in range(QS):
                    sl = slice(nn * NS, (nn + 1) * NS)
                    nc.vector.tensor_mul(qt[:, m, sl], qt[:, m, sl],
                                         rqb[:, sl])

        # =============== K projection (full S, feature-major) =============
        # nn-outer so only one ss sum tile is live at a time (sp bufs=3);
        # wk blocks are re-streamed per nn pass.  q_normalize pieces are
        # spread into the early passes so their DVE cost hides.
        kt = bigB.tile([P, DK, S], BF16, tag="bigB")
        wk_seq = [wblock(wk_pm, 0, nc.scalar), wblock(wk_pm, 1, nc.scalar)]
        kidx = 0
        sq_pass = []
        for nn in range(SNS):
            sl = slice(nn * NS, (nn + 1) * NS)
            ss_t = sp.tile([1, NS], F32, tag="sum", name=f"ss{nn}")
            for m in range(DK):
                if kidx + 2 < SNS * DK:
                    wk_seq.append(wblock(wk_pm, (kidx + 2) % DK, nc.scalar))
                wcur = wk_seq[kidx]
                if kidx == 6:
                    q_norm_bcast(0)
                elif kidx == 10:
                    q_norm_bcast(1)
                elif kidx == 14:
                    q_norm_mul()
                ps = pp.tile([P, NS], F32, tag="mm")
                for kk in range(DK):
                    nc.tensor.matmul(ps[:], wcur[:, kk, :], xt[:, kk, sl],
                                     start=(kk == 0), stop=(kk == DK - 1))
                nc.scalar.activation(kt[:, m, sl], ps[:], AF.Identity,
                                     bias=bkc[:, m:m + 1])
                sq = workp.tile([P, NS], BF16, tag="sq", bufs=10,
                                name=f"sqk{nn}_{m}")
                nc.vector.tensor_mul(sq[:], kt[:, m, sl], kt[:, m, sl])
                sq_pass.append(sq)
                kidx += 1
            # batched row-sum matmuls: one stationary transition per pass
            # instead of eight interleaved ones
            for m, sq in enumerate(sq_pass):
                nc.tensor.matmul(ss_t[:], ones_bf[:], sq[:],
                                 start=(m == 0), stop=(m == DK - 1))
            sq_pass.clear()
            # ||k_row|| for this slice as soon as its sums stop
            rk_row = workp.tile([1, NS], F32, tag="row", bufs=2)
            nc.scalar.activation(rk_row[:], ss_t[:], AF.Sqrt)
            nc.scalar.dma_start(out=rk_scr[0:1, nn * NS:(nn + 1) * NS],
                                in_=rk_row[:])
        # transpose to partitions, then reciprocal on [128, SK] (16/lane)
        rk_flat = rk_scr[0:1, :]
        nc.scalar.dma_start(
            out=rk_col[:],
            in_=bass.AP(tensor=rk_flat.tensor, offset=rk_flat.offset,
                        ap=[[1, P], [P, SK]]))
        nc.vector.reciprocal(rk_col[:], rk_col[:])

        # w1 blocks: own pool, all 8 resident well before MLP1
        w1_sb = [wblock(w1_pm, m, nc.scalar, pool=w1p, tag="w1")
                 for m in range(DK)]

        # =============== V projection (natural [s, d], x-stationary) ======
        v_sb = bigC.tile([P, SK, D], BF16, tag="bigC")
        wv_sl = []
        for dn in range(2):
            wv = oTp.tile([P, DK, NS], BF16, tag="oT", name=f"wv{dn}")
            nc.sync.dma_start(
                out=wv[:],
                in_=wvT[:, dn * NS:(dn + 1) * NS].rearrange(
                    "(kk p) n -> p kk n", p=P))
            wv_sl.append(wv)
        for dn in range(2):
            dsl = slice(dn * NS, (dn + 1) * NS)
            for st in range(SK):
                ps = pp.tile([P, NS], F32, tag="mm")
                for kk in range(DK):
                    nc.tensor.matmul(
                        ps[:], xt[:, kk, st * P:(st + 1) * P],
                        wv_sl[dn][:, kk, :],
                        start=(kk == 0), stop=(kk == DK - 1))
                nc.vector.scalar_tensor_tensor(
                    out=v_sb[:, st, dsl], in0=ps[:], scalar=1.0,
                    in1=bvb[:, dsl], op0=ALU.mult, op1=ALU.add)

        # =============== attention + MLP1 (interleaved issue) =============
        oTa = oTp.tile([P, DK // 2, QROWS], BF16, tag="oT", name="oTa")
        oTb = oTp.tile([P, DK // 2, QROWS], BF16, tag="oT", name="oTb")

        def oT(m, sl):
            t = oTa if m < DK // 2 else oTb
            return t[:, m % (DK // 2), sl]

        h1 = qtp.tile([P, DK, QROWS], BF16, tag="qtp", name="h1")

        def attention(qs):
            qsl = slice(qs * NS, (qs + 1) * NS)
            pt = workp.tile([P, SK, NS], BF16, tag="pt", bufs=1,
                            name=f"pt{qs}")
            s_ps = sp.tile([1, NS], F32, tag="sum", name=f"sden{qs}")
            for kkt in range(SK):
                ps = pp.tile([P, NS], F32, tag="mm")
                for kk in range(DK):
                    nc.tensor.matmul(
                        ps[:], kt[:, kk, kkt * P:(kkt + 1) * P],
                        qt[:, kk, qsl],
                        start=(kk == 0), stop=(kk == DK - 1))
                nc.scalar.activation(pt[:, kkt, :], ps[:], AF.Exp,
                                     scale=rk_col[:, kkt:kkt + 1])

            def pv_mms(m, pt=pt):
                po = pp.tile([P, NS], F32, tag="mm")
                for kkt in range(SK):
                    nc.tensor.matmul(po[:], v_sb[:, kkt, m * P:(m + 1) * P],
                                     pt[:, kkt, :],
                                     start=(kkt == 0), stop=(kkt == SK - 1))
                return po
            # PV m=0/1 don't need rsb -- only the DVE scale-out does -- so
            # the batched denominator sums, K=1 broadcast, and reciprocal
            # all hide under them.
            po0 = pv_mms(0)
            for kkt in range(SK):
                nc.tensor.matmul(s_ps[:], ones_bf[:], pt[:, kkt, :],
                                 start=(kkt == 0), stop=(kkt == SK - 1))
            po1 = pv_mms(1)
            s_rowr = workp.tile([1, NS], F32R, tag="rowr", bufs=2)
            nc.scalar.activation(s_rowr[:], s_ps[:], AF.Identity)
            pb = bp.tile([P, NS], F32, tag="bc")
            nc.tensor.matmul(pb[:], onesr[:], s_rowr[:], start=True,
                             stop=True)
            nc.vector.reciprocal(rsb[:, qs, :], pb[:])
            nc.vector.tensor_mul(oT(0, qsl), po0[:], rsb[:, qs, :])
            nc.vector.tensor_mul(oT(1, qsl), po1[:], rsb[:, qs, :])
            for m in range(2, DK):
                po = pv_mms(m)
                nc.vector.tensor_mul(oT(m, qsl), po[:], rsb[:, qs, :])

        def mlp1(nn):
            sl = slice(nn * NS, (nn + 1) * NS)
            for m in range(DK):
                ps = pp.tile([P, NS], F32, tag="mm")
                for kk in range(DK):
                    nc.tensor.matmul(ps[:], w1_sb[m][:, kk, :], oT(kk, sl),
                                     start=(kk == 0), stop=(kk == DK - 1))
                nc.scalar.activation(h1[:, m, sl], ps[:], AF.Relu,
                                     bias=b1c[:, m:m + 1])

        attention(0)
        attention(1)
        mlp1(0)
        mlp1(1)

        # =============== MLP2 (h2 resident bf16; W2 streamed per slice) ===
        # h2 halves land in the slots kt and xT vacated (same 32KB/part).
        h2a = bigB.tile([P, HK // 2, QROWS], BF16, tag="bigB", name="h2a")
        h2b = bigA.tile([P, HK // 2, QROWS], BF16, tag="bigA", name="h2b")

        def h2(ht, sl):
            t = h2a if ht < HK // 2 else h2b
            return t[:, ht % (HK // 2), sl]

        for nn in range(QS):
            sl = slice(nn * NS, (nn + 1) * NS)
            w2_sb = [wblock(w2_pm, 0, nc.sync), wblock(w2_pm, 1, nc.sync)]
            for ht in range(HK):
                if ht + 2 < HK:
                    w2_sb.append(wblock(w2_pm, ht + 2, nc.sync))
                wcur = w2_sb[ht]
                ps = pp.tile([P, NS], F32, tag="mm")
                for kk in range(DK):
                    nc.tensor.matmul(ps[:], wcur[:, kk, :], h1[:, kk, sl],
                                     start=(kk == 0), stop=(kk == DK - 1))
                nc.scalar.activation(h2(ht, sl), ps[:], AF.Relu,
                                     bias=b2c[:, ht:ht + 1])
                w2_sb[ht] = None

        # =============== MLP3 (feature-major out; host transposes) ========
        w3_sb = []

        def w3block(dt):
            w3t = oTp.tile([P, HK, P], BF16, tag="oT", name=f"w3b{dt}")
            nc.gpsimd.dma_start(
                out=w3t[:],
                in_=w3_pm[dt * P:(dt + 1) * P, :].rearrange(
                    "p (ht n) -> p ht n", ht=HK))
            return w3t

        w3_sb = [w3block(0), w3block(1)]
        for dt in range(DK):
            if dt + 2 < DK:
                w3_sb.append(w3block(dt + 2))
            wcur = w3_sb[dt]
            for nn in range(QS):
                sl = slice(nn * NS, (nn + 1) * NS)
                ps = pp.tile([P, NS], F32, tag="mm")
                for ht in range(HK):
                    nc.tensor.matmul(ps[:], wcur[:, ht, :], h2(ht, sl),
                                     start=(ht == 0), stop=(ht == HK - 1))
                ost = workp.tile([P, NS], F32, tag="ost", bufs=2)
                nc.scalar.activation(ost[:], ps[:], AF.Identity,
                                     bias=b3c[:, dt:dt + 1])
                nc.sync.dma_start(
                    out=out_pm[dt * P:(dt + 1) * P, sl], in_=ost[:])
            w3_sb[dt] = None

        for pool in (bp, sp, pp, workp, w1p, streamp, oTp, bigC,
                     bigB, qtp, bigA, constp, dram):
            pool.release()

    nc.compile()
    return nc


def _get_built():
    global _BUILT
    if _BUILT is None:
        _BUILT = _build()
    return _BUILT


def _pe_major(w, rows, cols):
    """[rows, cols] -> PE-major: block (m) holds lhsT [in-f part, out-f]."""
    return np.ascontiguousarray(
        w.reshape(rows // P, P, cols // P, P).transpose(2, 1, 0, 3)
        .reshape(cols, rows))


def _host_prep(inputs):
    import ml_dtypes
    bf16 = ml_dtypes.bfloat16
    f32 = np.float32

    def bf(a):
        return np.ascontiguousarray(np.asarray(a, f32).astype(bf16))

    x = np.asarray(inputs["x"], f32)
    shared = {
        "wq_pm": _pe_major(bf(inputs["Wq"]).T, D, D),
        "wk_pm": _pe_major(bf(inputs["Wk"]).T, D, D),
        "wvT": np.ascontiguousarray(bf(inputs["Wv"]).T),
        "w1_pm": _pe_major(bf(inputs["W1"]).T, D, D),
        "w2_pm": _pe_major(bf(inputs["W2"]).T, D, H),
        "w3_pm": _pe_major(bf(inputs["W3"]).T, H, D),
        "bq_col": np.ascontiguousarray(
            np.asarray(inputs["bq"], f32).reshape(DK, P).T),
        "bk_col": np.ascontiguousarray(
            np.asarray(inputs["bk"], f32).reshape(DK, P).T),
        "b1_col": np.ascontiguousarray(
            np.asarray(inputs["b1"], f32).reshape(DK, P).T),
        "b2_col": np.ascontiguousarray(
            np.asarray(inputs["b2"], f32).reshape(HK, P).T),
        "b3_col": np.ascontiguousarray(
            np.asarray(inputs["b3"], f32).reshape(DK, P).T),
        "bv_row": bf(inputs["bv"]).reshape(1, D),
    }
    in_maps = []
    for c in range(N_CORES):
        b, h = c // 2, c % 2
        m = dict(shared)
        xb = bf(x[b]).T  # [D, S]
        if h == 0:
            m["xTd"] = np.ascontiguousarray(xb)
        else:
            m["xTd"] = np.ascontiguousarray(
                np.concatenate([xb[:, QROWS:], xb[:, :QROWS]], axis=1))
        in_maps.append(m)
    return in_maps


def run_kernel(inputs, trace=False):
    """Returns (output [B,S,D] f32, exec_time_ns or None)."""
    from concourse.bass_utils import run_bass_kernel_spmd

    if trace:
        _install_ntff_hook()
    nc = _get_built()
    in_maps = _host_prep(inputs)
    res = run_bass_kernel_spmd(
        nc, in_maps, core_ids=list(range(N_CORES)), trace=trace)
    global _LAST_INSTS
    if res.instructions_and_trace is not None:
        _LAST_INSTS = res.instructions_and_trace[0]
    outp = np.empty((B, S, D), np.float32)
    for c in range(N_CORES):
        b, h = c // 2, c % 2
        outp[b, h * QROWS:(h + 1) * QROWS, :] = res.results[c]["out_pm"].T
    return outp, res.exec_time_ns


def kernel(**inputs):
    return run_kernel(inputs, trace=False)[0]


def _install_ntff_hook():
    """Register the axon NTFF profiling hook (used only when trace=True)."""
    import sys
    import types

    if "antenv.axon_hooks" in sys.modules:
        return
    try:
        import antenv
        from trn_agent_boot.trn_boot import _ntff_profile_via_ctypes
    except ImportError:
        return
    hooks = types.ModuleType("antenv.axon_hooks")
    _h = [_ntff_profile_via_ctypes("/opt/axon/libaxon_pjrt.so")]
    hooks.set_axon_ntff_profile_hook = lambda h: _h.__setitem__(0, h)
    hooks.get_axon_ntff_profile_hook = lambda: _h[0]
    sys.modules["antenv.axon_hooks"] = hooks
    antenv.axon_hooks = hooks
